# revision 45
# baseline (speedup 1.0000x reference)
"""BandSplit kernel for TRN2 (8 NeuronCores, SPMD data-parallel over tokens).

Reference computation (per band b of width w, D=384):
    xn = x_b / ||x_b||_2 * gamma_b * sqrt(w)
    y[..., b, :] = xn @ W_b + bias_b

Device strategy (per core, 512 of 4096 tokens):
  - gamma_b * sqrt(w) is folded into W_b on the host; bias is added on the
    host only if nonzero (it is all zeros for this problem's inputs).
  - 1/||x_b|| factors out of the matmul, so the kernel computes
    s = 1/sqrt(sum(x_b^2)) in token-major layout, runs the grouped GEMM on
    raw x (transposed on the PE), and applies s on the PSUM->SBUF eviction.
  - Matmul operands must start at partition 0/32/64, so bands are packed
    into 32-aligned "slots" of 128-partition chunks. The PE transpose does
    this repacking for free (its PSUM output lands at the slot base); the
    weight matrix is padded host-side into the mirrored layout.
"""

import math
from contextlib import ExitStack

import numpy as np

import concourse.bass as bass
import concourse.tile as tile
from concourse import bacc, mybir
from concourse.bass_utils import run_bass_kernel_spmd
from concourse.masks import make_identity

# ---------------------------------------------------------------- constants
BANDS = [8] * 24 + [16] * 12 + [48] * 8 + [96] * 8 + [192] * 8 + [512, 516]
NB = len(BANDS)            # 62
F = sum(BANDS)             # 4100
DIM = 384
B, T = 4, 1024
NTOK = B * T               # 4096
NCORES = 8
TOK_PER_CORE = NTOK // NCORES   # 512
P = 128
NTILE = TOK_PER_CORE // P       # 4 token tiles per core

_off = 0
OFFSETS = []
for _w in BANDS:
    OFFSETS.append(_off)
    _off += _w

# ---------------------------------------------------------------- layout
# Pack every band into 32-aligned slots of 128-partition chunks.
# A slot is (chunk, base, ln, feat_off); matmul lhsT/rhs base partition must
# be one of {0, 32, 64} and: ln<=32 -> base in {0,32,64}; ln<=64 -> {0,64};
# ln>64 -> base 0.
SEGS = [[] for _ in range(NB)]  # per band: list of (chunk, base, ln, feat)
_chunk = 0


def _alloc_chunks(band_ids, per_chunk, bases, ln_of, stride=None):
    # Bands are packed so a band placed at base 32/64 always has feat >= base
    # (required for the windowed transpose: its PSUM output must start at
    # partition 0, so the input window starts `base` features early).
    # When `stride` is given, the bands sharing a chunk are spaced so that
    # their features sit exactly 32 apart — then feat_j - base_j is constant
    # across the chunk and ONE windowed transpose serves all of them.
    global _chunk
    nchunks = (len(band_ids) + per_chunk - 1) // per_chunk
    for k in range(nchunks):
        if stride is None:
            group = band_ids[k::nchunks]
        else:
            g0 = (k // stride) * per_chunk * stride + (k % stride)
            group = [band_ids[g0 + stride * j] for j in range(per_chunk)]
        for j, b in enumerate(group):
            assert OFFSETS[b] >= bases[j], (b, bases[j])
            SEGS[b].append((_chunk + k, bases[j], ln_of(b), OFFSETS[b]))
    _chunk += nchunks


_w8 = [b for b in range(NB) if BANDS[b] == 8]
_w16 = [b for b in range(NB) if BANDS[b] == 16]
_w48 = [b for b in range(NB) if BANDS[b] == 48]
_w96 = [b for b in range(NB) if BANDS[b] == 96]
_w192 = [b for b in range(NB) if BANDS[b] == 192]
_big = [b for b in range(NB) if BANDS[b] > 192]

_alloc_chunks(_w8, 3, (0, 32, 64), lambda b: 8, stride=4)   # 32/8
_alloc_chunks(_w16, 3, (0, 32, 64), lambda b: 16, stride=2)  # 32/16
_alloc_chunks(_w48, 2, (0, 64), lambda b: 48)
_alloc_chunks(_w96, 1, (0,), lambda b: 96)
# w192: one full chunk + a 64-row tail, tails packed two per chunk
for _b in _w192:
    SEGS[_b].append((_chunk, 0, 128, OFFSETS[_b]))
    _chunk += 1
for _i in range(0, len(_w192), 2):
    for _j, _b in enumerate(_w192[_i : _i + 2]):
        SEGS[_b].append((_chunk, 64 * _j, 64, OFFSETS[_b] + 128))
    _chunk += 1
# w512 / w516: full chunks (+ a 4-row tail for 516)
for _b in _big:
    _w = BANDS[_b]
    _f = OFFSETS[_b]
    while _w > 0:
        ln = min(128, _w)
        SEGS[_b].append((_chunk, 0, ln, _f))
        _chunk += 1
        _f += ln
        _w -= ln

NCHUNK = _chunk  # 49

# per chunk: transpose jobs. Slots whose window origin (feat - base) agrees
# are served by ONE windowed transpose (rows [0, base+ln) from origin).
# All jobs of a chunk share one PSUM tile: a job writes pt[0, rows), so
# issuing in descending-rows order leaves every band's rows [base, base+ln)
# intact and the whole range [0, hi) initialized -> one PSUM->SBUF copy
# per chunk. TJOBS entries are (origin_feat, rows).
CHUNK_JOBS = [[] for _ in range(NCHUNK)]
for _b in range(NB):
    for (c, base, ln, feat) in SEGS[_b]:
        CHUNK_JOBS[c].append((base, ln, feat))
CHUNK_TJOBS = []
for _jobs in CHUNK_JOBS:
    byorg = {}
    for base, ln, feat in _jobs:
        org = feat - base
        byorg[org] = max(byorg.get(org, 0), base + ln)
    CHUNK_TJOBS.append(sorted(byorg.items(), key=lambda kv: -kv[1]))
CHUNK_HI = [max(base + ln for base, ln, _ in jobs) for jobs in CHUNK_JOBS]
N_TRANSPOSES = sum(len(j) for j in CHUNK_TJOBS)  # 53 per token tile

# band segs sorted by chunk so matmuls run in chunk order
BAND_ORDER = []  # (chunk of first seg, band) -> issue order
for _b in range(NB):
    SEGS[_b].sort()

# equal-width runs for grouped sum-of-squares reductions:
# (feat_start, n_bands, width, band0)
RGROUPS = [
    (0, 24, 8, 0),
    (192, 12, 16, 24),
    (384, 8, 48, 36),
    (768, 8, 96, 44),
    (1536, 8, 192, 52),
    (3072, 1, 512, 60),
    (3584, 1, 516, 61),
]

# output staging groups: bands packed per DMA (contiguous in y's band axis)
OUT_GROUP = 8
OGROUPS = [(g, min(g + OUT_GROUP, NB)) for g in range(0, NB, OUT_GROUP)]

F32 = mybir.dt.float32
# Matmul operand dtype: float32r streams at full PE rate for N>=256 (fp32
# is 4 cycles/row). Operand-producing instructions must write the dtype
# themselves (BIR verifier rule), so the wt/xc tiles are declared fp32r.
MM_DT = mybir.dt.float32r


def _build_body(ctx: ExitStack, tc: tile.TileContext, x_ap, w_ap, y_ap):
    nc = tc.nc

    const = ctx.enter_context(tc.tile_pool(name="const", bufs=1))
    wpool = ctx.enter_context(tc.tile_pool(name="wpool", bufs=1))
    xpool = ctx.enter_context(tc.tile_pool(name="xpool", bufs=2))
    x2pool = ctx.enter_context(tc.tile_pool(name="x2pool", bufs=2))
    xtpool = ctx.enter_context(tc.tile_pool(name="xtpool", bufs=6))
    stats = ctx.enter_context(tc.tile_pool(name="stats", bufs=2))
    outpool = ctx.enter_context(tc.tile_pool(name="outpool", bufs=2))
    psum_tp = ctx.enter_context(tc.tile_pool(name="psum_tp", bufs=3, space="PSUM"))
    psum_mm = ctx.enter_context(tc.tile_pool(name="psum_mm", bufs=5, space="PSUM"))

    identf = const.tile([P, P], F32, tag="identf")
    make_identity(nc, identf[:])
    ident = const.tile([P, P], MM_DT, tag="ident")
    nc.vector.tensor_copy(ident[:], identf[:])

    # weights, slot-layout in DRAM already: w_ap is (NCHUNK*128, 384);
    # SBUF wt[p, c*DIM + d] = wpad[c*128 + p, d]. Issued on the gpsimd
    # (SWDGE) queue so the bulk load streams in parallel with the
    # latency-critical first x-tile load on the sync HWDGE queue.
    wt = wpool.tile([P, NCHUNK * DIM], MM_DT, tag="wt")
    # Runs of equal-height chunks load as one DMA trimmed to rows [0, hi):
    # skips ~2.4MB of slot-padding traffic with only ~6 DMA instructions
    # (more would starve the strict-FIFO gpsimd queue and delay the
    # squares). Early chunks arrive early for the first bands' matmuls.
    _c = 0
    while _c < NCHUNK:
        hi = CHUNK_HI[_c]
        _c2 = _c
        while _c2 < NCHUNK and CHUNK_HI[_c2] == hi and _c2 - _c < 7:
            _c2 += 1
        nc.gpsimd.dma_start(
            wt[:hi, _c * DIM : _c2 * DIM].rearrange("p (c d) -> p c d", d=DIM),
            w_ap[_c * P : _c2 * P, :].rearrange("(c p) d -> p c d", p=P)[:hi],
        )
        _c = _c2

    # FS = 1536 is exactly the small-band/big-band chunk boundary: x rows
    # are loaded in two halves so the small-band transposes and stats start
    # after the first ~0.8MB lands. Stats are likewise split into a
    # small-band half (bands 0-51, needs only xta) and a big-band half, so
    # the first evictions only wait for the small, early half. The stats
    # phase of tile i+1 is emitted BEFORE the compute phase of tile i so
    # its reduces run ahead of tile i's eviction flood in the DVE queue.
    FS = 1536
    NBA = 52  # bands 0-51 live entirely in features [0, FS)

    def stats_phase(it):
        t0 = it * P
        xta = xpool.tile([P, FS], MM_DT, tag="xta")
        # two dma_starts land on different HWDGE queues -> parallel streams
        nc.sync.dma_start(xta[:, : FS // 2], x_ap[t0 : t0 + P, : FS // 2])
        nc.sync.dma_start(xta[:, FS // 2 :], x_ap[t0 : t0 + P, FS // 2 : FS])
        xtb = xpool.tile([P, F - FS], MM_DT, tag="xtb")
        nc.sync.dma_start(xtb[:], x_ap[t0 : t0 + P, FS:])

        # tile 0's squares go on ACT (idle during the ramp and much faster
        # than gpsimd); later tiles run during the previous tile's compute,
        # where the otherwise-idle gpsimd has free capacity
        x2 = x2pool.tile([P, F], F32, tag="x2")
        if it == 0:
            # DVE, not ACT: the ACT FIFO would serialize these ahead of the
            # first quad copies and evictions during the ramp
            nc.vector.tensor_tensor(
                x2[:, :FS], xta[:], xta[:], op=mybir.AluOpType.mult
            )
            nc.vector.tensor_tensor(
                x2[:, FS:], xtb[:], xtb[:], op=mybir.AluOpType.mult
            )
        else:
            nc.gpsimd.tensor_tensor(
                x2[:, :FS], xta[:], xta[:], op=mybir.AluOpType.mult
            )
            nc.gpsimd.tensor_tensor(
                x2[:, FS:], xtb[:], xtb[:], op=mybir.AluOpType.mult
            )
        qqa = stats.tile([P, NBA], F32, tag="qqa")
        qqb = stats.tile([P, NB - NBA], F32, tag="qqb")
        for fs, nb, wd, b0 in RGROUPS:
            dst = qqa[:, b0 : b0 + nb] if b0 < NBA else qqb[:, b0 - NBA : b0 - NBA + nb]
            nc.vector.reduce_sum(
                dst,
                x2[:, fs : fs + nb * wd].rearrange("p (n w) -> p n w", w=wd),
                axis=mybir.AxisListType.X,
            )
        nrma = stats.tile([P, NBA], F32, tag="nrma")
        nc.scalar.sqrt(nrma[:], qqa[:])
        sca = stats.tile([P, NBA], F32, tag="sca")
        nc.vector.reciprocal(sca[:], nrma[:])
        nrmb = stats.tile([P, NB - NBA], F32, tag="nrmb")
        nc.scalar.sqrt(nrmb[:], qqb[:])
        scb = stats.tile([P, NB - NBA], F32, tag="scb")
        nc.vector.reciprocal(scb[:], nrmb[:])
        return xta, xtb, sca, scb

    phase = [None] * NTILE
    phase[0] = stats_phase(0)
    for it in range(NTILE):
        t0 = it * P
        if it + 1 < NTILE:
            phase[it + 1] = stats_phase(it + 1)
        xta, xtb, sca, scb = phase[it]

        def xwin(org, rows):
            if org + rows <= FS:
                return xta[:, org : org + rows]
            assert org >= FS, (org, rows)
            return xtb[:, org - FS : org - FS + rows]

        # ---- transpose x into 32-aligned slot chunks: xc[base+r, t].
        # Transpose-matmul PSUM outputs must start at partition 0, so each
        # job's input window starts `base` features early; descending-base
        # issue order within the shared pt tile leaves every band's rows
        # intact and [0, hi) fully initialized -> single copy per chunk.
        # 4 chunks share one PSUM bank ([128, 512] f32) and one SBUF tile,
        # so a single wide copy evicts 4 chunks (DVE cost scales with the
        # free dim, so one [hi, 512] copy beats four [hi, 128] copies).
        xcs = [None] * NCHUNK
        xcoff = [None] * NCHUNK
        for q0 in range(0, NCHUNK, 4):
            q1 = min(q0 + 4, NCHUNK)
            xc = xtpool.tile([P, 4 * P], MM_DT, tag="xc")
            pt = psum_tp.tile([P, 4 * P], MM_DT, tag="pt")
            for c in range(q0, q1):
                col = (c - q0) * P
                for org, rows in CHUNK_TJOBS[c]:
                    nc.tensor.transpose(
                        pt[:rows, col : col + P], xwin(org, rows), ident[:]
                    )
            qhi = max(CHUNK_HI[c] for c in range(q0, q1))
            if (q0 // 4) % 2 == 0:
                nc.vector.tensor_copy(
                    xc[:qhi, : (q1 - q0) * P], pt[:qhi, : (q1 - q0) * P]
                )
            else:
                nc.scalar.copy(
                    xc[:qhi, : (q1 - q0) * P], pt[:qhi, : (q1 - q0) * P]
                )
            for c in range(q0, q1):
                xcs[c] = xc
                xcoff[c] = (c - q0) * P

        # ---- grouped GEMM + scaled eviction, bands staged in output groups
        for g0, g1 in OGROUPS:
            go = outpool.tile([P, OUT_GROUP * DIM], F32, tag="go")
            for b in range(g0, g1):
                pm = psum_mm.tile([P, DIM], F32, tag="pm")
                segs = SEGS[b]
                for si, (c, base, ln, feat) in enumerate(segs):
                    co = xcoff[c]
                    lhsT = xcs[c][base : base + ln, co : co + P]
                    rhs = wt[base : base + ln, c * DIM : (c + 1) * DIM]
                    nc.tensor.matmul(
                        pm[:],
                        lhsT,
                        rhs,
                        start=(si == 0),
                        stop=(si == len(segs) - 1),
                    )
                dst = go[:, (b - g0) * DIM : (b - g0 + 1) * DIM]
                scp = sca[:, b : b + 1] if b < NBA else scb[:, b - NBA : b - NBA + 1]
                if b % 9 < 4:  # DVE also owns copies/reduces; tilt to ACT
                    nc.vector.tensor_scalar_mul(dst, pm[:], scp)
                else:
                    nc.scalar.mul(dst, pm[:], scp)
            nc.sync.dma_start(
                y_ap[t0 : t0 + P, g0 * DIM : g1 * DIM],
                go[:, : (g1 - g0) * DIM],
            )


_cached_nc = None


def _build():
    global _cached_nc
    if _cached_nc is not None:
        return _cached_nc
    nc = bacc.Bacc(
        "TRN2",
        target_bir_lowering=False,
        debug=False,
        enable_asserts=False,
        num_devices=NCORES,
    )
    x_ap = nc.dram_tensor("x", (TOK_PER_CORE, F), MM_DT, kind="ExternalInput").ap()
    w_ap = nc.dram_tensor("w", (NCHUNK * P, DIM), MM_DT, kind="ExternalInput").ap()
    y_ap = nc.dram_tensor(
        "y", (TOK_PER_CORE, NB * DIM), F32, kind="ExternalOutput"
    ).ap()
    with tile.TileContext(nc) as tc:
        with ExitStack() as ctx:
            _build_body(ctx, tc, x_ap, w_ap, y_ap)
    nc.compile()
    _cached_nc = nc
    return nc


def _host_prep(x, gammas, weights, biases):
    x = np.ascontiguousarray(np.asarray(x, np.float32)).reshape(NTOK, F)
    wpad = np.zeros((NCHUNK * P, DIM), np.float32)
    for b in range(NB):
        g = np.asarray(gammas[b], np.float32)
        w = np.asarray(weights[b], np.float32)
        wb = w * (g * np.float32(math.sqrt(len(g))))[:, None]
        for (c, base, ln, feat) in SEGS[b]:
            lo = OFFSETS[b]
            wpad[c * P + base : c * P + base + ln] = wb[feat - lo : feat - lo + ln]
    bias = np.stack([np.asarray(bb, np.float32) for bb in biases], axis=0)
    return x, wpad, bias


def kernel(x, gammas, weights, biases, _trace=False):
    xf, wpad, bias = _host_prep(x, gammas, weights, biases)
    nc = _build()
    in_maps = [
        {"x": xf[c * TOK_PER_CORE : (c + 1) * TOK_PER_CORE], "w": wpad}
        for c in range(NCORES)
    ]
    res = run_bass_kernel_spmd(
        nc, in_maps, core_ids=list(range(NCORES)), trace=_trace
    )
    y = np.concatenate([r["y"] for r in res.results], axis=0)
    y = y.reshape(B, T, NB, DIM)
    if np.any(bias):
        y = y + bias[None, None]
    if _trace:
        return y, res
    return y


# revision 47
# speedup vs baseline: 1.0425x; 1.0425x over previous
"""BandSplit kernel for TRN2 (8 NeuronCores, SPMD data-parallel over tokens).

Reference computation (per band b of width w, D=384):
    xn = x_b / ||x_b||_2 * gamma_b * sqrt(w)
    y[..., b, :] = xn @ W_b + bias_b

Device strategy (per core, 512 of 4096 tokens):
  - gamma_b * sqrt(w) is folded into W_b on the host; bias is added on the
    host only if nonzero (it is all zeros for this problem's inputs).
  - 1/||x_b|| factors out of the matmul, so the kernel computes
    s = 1/sqrt(sum(x_b^2)) in token-major layout, runs the grouped GEMM on
    raw x (transposed on the PE), and applies s on the PSUM->SBUF eviction.
  - Matmul operands must start at partition 0/32/64, so bands are packed
    into 32-aligned "slots" of 128-partition chunks. The PE transpose does
    this repacking for free (its PSUM output lands at the slot base); the
    weight matrix is padded host-side into the mirrored layout.
"""

import math
from contextlib import ExitStack

import numpy as np

import concourse.tile as tile
from concourse import bacc, mybir
from concourse.bass_utils import run_bass_kernel_spmd
from concourse.masks import make_identity

# ---------------------------------------------------------------- constants
BANDS = [8] * 24 + [16] * 12 + [48] * 8 + [96] * 8 + [192] * 8 + [512, 516]
NB = len(BANDS)            # 62
F = sum(BANDS)             # 4100
DIM = 384
B, T = 4, 1024
NTOK = B * T               # 4096
NCORES = 8
TOK_PER_CORE = NTOK // NCORES   # 512
P = 128
NTILE = TOK_PER_CORE // P       # 4 token tiles per core

_off = 0
OFFSETS = []
for _w in BANDS:
    OFFSETS.append(_off)
    _off += _w

# ---------------------------------------------------------------- layout
# Pack every band into 32-aligned slots of 128-partition chunks.
# A slot is (chunk, base, ln, feat_off); matmul lhsT/rhs base partition must
# be one of {0, 32, 64} and: ln<=32 -> base in {0,32,64}; ln<=64 -> {0,64};
# ln>64 -> base 0.
SEGS = [[] for _ in range(NB)]  # per band: list of (chunk, base, ln, feat)
_chunk = 0


def _alloc_chunks(band_ids, per_chunk, bases, ln_of, stride=None):
    # Bands are packed so a band placed at base 32/64 always has feat >= base
    # (required for the windowed transpose: its PSUM output must start at
    # partition 0, so the input window starts `base` features early).
    # When `stride` is given, the bands sharing a chunk are spaced so that
    # their features sit exactly 32 apart — then feat_j - base_j is constant
    # across the chunk and ONE windowed transpose serves all of them.
    global _chunk
    nchunks = (len(band_ids) + per_chunk - 1) // per_chunk
    for k in range(nchunks):
        if stride is None:
            group = band_ids[k::nchunks]
        else:
            g0 = (k // stride) * per_chunk * stride + (k % stride)
            group = [band_ids[g0 + stride * j] for j in range(per_chunk)]
        for j, b in enumerate(group):
            assert OFFSETS[b] >= bases[j], (b, bases[j])
            SEGS[b].append((_chunk + k, bases[j], ln_of(b), OFFSETS[b]))
    _chunk += nchunks


_w8 = [b for b in range(NB) if BANDS[b] == 8]
_w16 = [b for b in range(NB) if BANDS[b] == 16]
_w48 = [b for b in range(NB) if BANDS[b] == 48]
_w96 = [b for b in range(NB) if BANDS[b] == 96]
_w192 = [b for b in range(NB) if BANDS[b] == 192]
_big = [b for b in range(NB) if BANDS[b] > 192]

_alloc_chunks(_w8, 3, (0, 32, 64), lambda b: 8, stride=4)   # 32/8
_alloc_chunks(_w16, 3, (0, 32, 64), lambda b: 16, stride=2)  # 32/16
_alloc_chunks(_w48, 2, (0, 64), lambda b: 48)
_alloc_chunks(_w96, 1, (0,), lambda b: 96)
# w192: one full chunk + a 64-row tail, tails packed two per chunk
for _b in _w192:
    SEGS[_b].append((_chunk, 0, 128, OFFSETS[_b]))
    _chunk += 1
for _i in range(0, len(_w192), 2):
    for _j, _b in enumerate(_w192[_i : _i + 2]):
        SEGS[_b].append((_chunk, 64 * _j, 64, OFFSETS[_b] + 128))
    _chunk += 1
# w512 / w516: full chunks (+ a 4-row tail for 516)
for _b in _big:
    _w = BANDS[_b]
    _f = OFFSETS[_b]
    while _w > 0:
        ln = min(128, _w)
        SEGS[_b].append((_chunk, 0, ln, _f))
        _chunk += 1
        _f += ln
        _w -= ln

NCHUNK = _chunk  # 49

# per chunk: transpose jobs. Slots whose window origin (feat - base) agrees
# are served by ONE windowed transpose (rows [0, base+ln) from origin).
# All jobs of a chunk share one PSUM tile: a job writes pt[0, rows), so
# issuing in descending-rows order leaves every band's rows [base, base+ln)
# intact and the whole range [0, hi) initialized -> one PSUM->SBUF copy
# per chunk. TJOBS entries are (origin_feat, rows).
CHUNK_JOBS = [[] for _ in range(NCHUNK)]
for _b in range(NB):
    for (c, base, ln, feat) in SEGS[_b]:
        CHUNK_JOBS[c].append((base, ln, feat))
CHUNK_TJOBS = []
for _jobs in CHUNK_JOBS:
    byorg = {}
    for base, ln, feat in _jobs:
        org = feat - base
        byorg[org] = max(byorg.get(org, 0), base + ln)
    CHUNK_TJOBS.append(sorted(byorg.items(), key=lambda kv: -kv[1]))
CHUNK_HI = [max(base + ln for base, ln, _ in jobs) for jobs in CHUNK_JOBS]
N_TRANSPOSES = sum(len(j) for j in CHUNK_TJOBS)  # 53 per token tile

# band segs sorted by chunk so matmuls run in chunk order
for _b in range(NB):
    SEGS[_b].sort()

# equal-width runs for grouped sum-of-squares reductions:
# (feat_start, n_bands, width, band0)
RGROUPS = [
    (0, 24, 8, 0),
    (192, 12, 16, 24),
    (384, 8, 48, 36),
    (768, 8, 96, 44),
    (1536, 8, 192, 52),
    (3072, 1, 512, 60),
    (3584, 1, 516, 61),
]

# output staging groups: bands packed per DMA (contiguous in y's band axis)
OUT_GROUP = 8
OGROUPS = [(g, min(g + OUT_GROUP, NB)) for g in range(0, NB, OUT_GROUP)]

F32 = mybir.dt.float32
# Matmul operand dtype: float32r streams at full PE rate for N>=256 (fp32
# is 4 cycles/row). Operand-producing instructions must write the dtype
# themselves (BIR verifier rule), so the wt/xc tiles are declared fp32r.
MM_DT = mybir.dt.float32r


def _build_body(ctx: ExitStack, tc: tile.TileContext, x_ap, w_ap, y_ap):
    nc = tc.nc

    const = ctx.enter_context(tc.tile_pool(name="const", bufs=1))
    wpool = ctx.enter_context(tc.tile_pool(name="wpool", bufs=1))
    xpool = ctx.enter_context(tc.tile_pool(name="xpool", bufs=2))
    x2pool = ctx.enter_context(tc.tile_pool(name="x2pool", bufs=2))
    xtpool = ctx.enter_context(tc.tile_pool(name="xtpool", bufs=6))
    stats = ctx.enter_context(tc.tile_pool(name="stats", bufs=2))
    outpool = ctx.enter_context(tc.tile_pool(name="outpool", bufs=2))
    psum_tp = ctx.enter_context(tc.tile_pool(name="psum_tp", bufs=3, space="PSUM"))
    psum_mm = ctx.enter_context(tc.tile_pool(name="psum_mm", bufs=5, space="PSUM"))

    identf = const.tile([P, P], F32, tag="identf")
    make_identity(nc, identf[:])
    ident = const.tile([P, P], MM_DT, tag="ident")
    nc.vector.tensor_copy(ident[:], identf[:])

    # weights, slot-layout in DRAM already: w_ap is (NCHUNK*128, 384);
    # SBUF wt[p, c*DIM + d] = wpad[c*128 + p, d]. Issued on the gpsimd
    # (SWDGE) queue so the bulk load streams in parallel with the
    # latency-critical first x-tile load on the sync HWDGE queue.
    wt = wpool.tile([P, NCHUNK * DIM], MM_DT, tag="wt")
    # Runs of equal-height chunks load as one DMA trimmed to rows [0, hi):
    # skips ~2.4MB of slot-padding traffic with only ~6 DMA instructions
    # (more would starve the strict-FIFO gpsimd queue and delay the
    # squares). Early chunks arrive early for the first bands' matmuls.
    _c = 0
    while _c < NCHUNK:
        hi = CHUNK_HI[_c]
        _c2 = _c
        while _c2 < NCHUNK and CHUNK_HI[_c2] == hi and _c2 - _c < 7:
            _c2 += 1
        nc.gpsimd.dma_start(
            wt[:hi, _c * DIM : _c2 * DIM].rearrange("p (c d) -> p c d", d=DIM),
            w_ap[_c * P : _c2 * P, :].rearrange("(c p) d -> p c d", p=P)[:hi],
        )
        _c = _c2

    FS = 1536
    for it in range(NTILE):
        t0 = it * P
        xta = xpool.tile([P, FS], MM_DT, tag="xta")
        nc.sync.dma_start(xta[:], x_ap[t0 : t0 + P, :FS])
        xtb = xpool.tile([P, F - FS], MM_DT, tag="xtb")
        nc.sync.dma_start(xtb[:], x_ap[t0 : t0 + P, FS:])

        x2 = x2pool.tile([P, F], F32, tag="x2")
        nc.gpsimd.tensor_tensor(x2[:, :FS], xta[:], xta[:], op=mybir.AluOpType.mult)
        nc.gpsimd.tensor_tensor(x2[:, FS:], xtb[:], xtb[:], op=mybir.AluOpType.mult)
        qq = stats.tile([P, NB], F32, tag="qq")
        for fs, nb, wd, b0 in RGROUPS:
            nc.vector.reduce_sum(
                qq[:, b0 : b0 + nb],
                x2[:, fs : fs + nb * wd].rearrange("p (n w) -> p n w", w=wd),
                axis=mybir.AxisListType.X,
            )
        nrm = stats.tile([P, NB], F32, tag="nrm")
        nc.scalar.sqrt(nrm[:], qq[:])
        sc = stats.tile([P, NB], F32, tag="sc")
        nc.vector.reciprocal(sc[:], nrm[:])

        def xwin(org, rows):
            if org + rows <= FS:
                return xta[:, org : org + rows]
            assert org >= FS, (org, rows)
            return xtb[:, org - FS : org - FS + rows]

        # ---- transpose x into 32-aligned slot chunks: xc[base+r, t].
        # Transpose-matmul PSUM outputs must start at partition 0, so each
        # job's input window starts `base` features early; descending-base
        # issue order within the shared pt tile leaves every band's rows
        # intact and [0, hi) fully initialized -> single copy per chunk.
        # 4 chunks share one PSUM bank ([128, 512] f32) and one SBUF tile,
        # so a single wide copy evicts 4 chunks (DVE cost scales with the
        # free dim, so one [hi, 512] copy beats four [hi, 128] copies).
        xcs = [None] * NCHUNK
        xcoff = [None] * NCHUNK
        for q0 in range(0, NCHUNK, 4):
            q1 = min(q0 + 4, NCHUNK)
            xc = xtpool.tile([P, 4 * P], MM_DT, tag="xc")
            pt = psum_tp.tile([P, 4 * P], MM_DT, tag="pt")
            for c in range(q0, q1):
                col = (c - q0) * P
                for org, rows in CHUNK_TJOBS[c]:
                    nc.tensor.transpose(
                        pt[:rows, col : col + P], xwin(org, rows), ident[:]
                    )
            qhi = max(CHUNK_HI[c] for c in range(q0, q1))
            if (q0 // 4) % 2 == 0:
                nc.vector.tensor_copy(
                    xc[:qhi, : (q1 - q0) * P], pt[:qhi, : (q1 - q0) * P]
                )
            else:
                nc.scalar.copy(
                    xc[:qhi, : (q1 - q0) * P], pt[:qhi, : (q1 - q0) * P]
                )
            for c in range(q0, q1):
                xcs[c] = xc
                xcoff[c] = (c - q0) * P

        # ---- grouped GEMM + scaled eviction, bands staged in output groups
        for g0, g1 in OGROUPS:
            go = outpool.tile([P, OUT_GROUP * DIM], F32, tag="go")
            for b in range(g0, g1):
                pm = psum_mm.tile([P, DIM], F32, tag="pm")
                segs = SEGS[b]
                for si, (c, base, ln, feat) in enumerate(segs):
                    co = xcoff[c]
                    lhsT = xcs[c][base : base + ln, co : co + P]
                    rhs = wt[base : base + ln, c * DIM : (c + 1) * DIM]
                    nc.tensor.matmul(
                        pm[:],
                        lhsT,
                        rhs,
                        start=(si == 0),
                        stop=(si == len(segs) - 1),
                    )
                dst = go[:, (b - g0) * DIM : (b - g0 + 1) * DIM]
                scp = sc[:, b : b + 1]
                if b % 9 < 4:  # DVE also owns copies/reduces; tilt to ACT
                    nc.vector.tensor_scalar_mul(dst, pm[:], scp)
                else:
                    nc.scalar.mul(dst, pm[:], scp)
            nc.sync.dma_start(
                y_ap[t0 : t0 + P, g0 * DIM : g1 * DIM],
                go[:, : (g1 - g0) * DIM],
            )


_cached_nc = None


def _build():
    global _cached_nc
    if _cached_nc is not None:
        return _cached_nc
    nc = bacc.Bacc(
        "TRN2",
        target_bir_lowering=False,
        debug=False,
        enable_asserts=False,
        num_devices=NCORES,
    )
    x_ap = nc.dram_tensor("x", (TOK_PER_CORE, F), MM_DT, kind="ExternalInput").ap()
    w_ap = nc.dram_tensor("w", (NCHUNK * P, DIM), MM_DT, kind="ExternalInput").ap()
    y_ap = nc.dram_tensor(
        "y", (TOK_PER_CORE, NB * DIM), F32, kind="ExternalOutput"
    ).ap()
    with tile.TileContext(nc) as tc:
        with ExitStack() as ctx:
            _build_body(ctx, tc, x_ap, w_ap, y_ap)
    nc.compile()
    _cached_nc = nc
    return nc


def _host_prep(x, gammas, weights, biases):
    x = np.ascontiguousarray(np.asarray(x, np.float32)).reshape(NTOK, F)
    wpad = np.zeros((NCHUNK * P, DIM), np.float32)
    for b in range(NB):
        g = np.asarray(gammas[b], np.float32)
        w = np.asarray(weights[b], np.float32)
        wb = w * (g * np.float32(math.sqrt(len(g))))[:, None]
        for (c, base, ln, feat) in SEGS[b]:
            lo = OFFSETS[b]
            wpad[c * P + base : c * P + base + ln] = wb[feat - lo : feat - lo + ln]
    bias = np.stack([np.asarray(bb, np.float32) for bb in biases], axis=0)
    return x, wpad, bias


def kernel(x, gammas, weights, biases, _trace=False):
    xf, wpad, bias = _host_prep(x, gammas, weights, biases)
    nc = _build()
    in_maps = [
        {"x": xf[c * TOK_PER_CORE : (c + 1) * TOK_PER_CORE], "w": wpad}
        for c in range(NCORES)
    ]
    res = run_bass_kernel_spmd(
        nc, in_maps, core_ids=list(range(NCORES)), trace=_trace
    )
    y = np.concatenate([r["y"] for r in res.results], axis=0)
    y = y.reshape(B, T, NB, DIM)
    if np.any(bias):
        y = y + bias[None, None]
    if _trace:
        return y, res
    return y


# revision 48
# speedup vs baseline: 1.0599x; 1.0167x over previous
"""BandSplit kernel for TRN2 (8 NeuronCores, SPMD data-parallel over tokens).

Reference computation (per band b of width w, D=384):
    xn = x_b / ||x_b||_2 * gamma_b * sqrt(w)
    y[..., b, :] = xn @ W_b + bias_b

Device strategy (per core, 512 of 4096 tokens):
  - gamma_b * sqrt(w) is folded into W_b on the host; bias is added on the
    host only if nonzero (it is all zeros for this problem's inputs).
  - 1/||x_b|| factors out of the matmul, so the kernel computes
    s = 1/sqrt(sum(x_b^2)) in token-major layout, runs the grouped GEMM on
    raw x (transposed on the PE), and applies s on the PSUM->SBUF eviction.
  - Matmul operands must start at partition 0/32/64, so bands are packed
    into 32-aligned "slots" of 128-partition chunks. The PE transpose does
    this repacking for free (its PSUM output lands at the slot base); the
    weight matrix is padded host-side into the mirrored layout.
"""

import math
from contextlib import ExitStack

import numpy as np

import concourse.tile as tile
from concourse import bacc, mybir
from concourse.bass_utils import run_bass_kernel_spmd
from concourse.masks import make_identity

# ---------------------------------------------------------------- constants
BANDS = [8] * 24 + [16] * 12 + [48] * 8 + [96] * 8 + [192] * 8 + [512, 516]
NB = len(BANDS)            # 62
F = sum(BANDS)             # 4100
DIM = 384
B, T = 4, 1024
NTOK = B * T               # 4096
NCORES = 8
TOK_PER_CORE = NTOK // NCORES   # 512
P = 128
NTILE = TOK_PER_CORE // P       # 4 token tiles per core

_off = 0
OFFSETS = []
for _w in BANDS:
    OFFSETS.append(_off)
    _off += _w

# ---------------------------------------------------------------- layout
# Pack every band into 32-aligned slots of 128-partition chunks.
# A slot is (chunk, base, ln, feat_off); matmul lhsT/rhs base partition must
# be one of {0, 32, 64} and: ln<=32 -> base in {0,32,64}; ln<=64 -> {0,64};
# ln>64 -> base 0.
SEGS = [[] for _ in range(NB)]  # per band: list of (chunk, base, ln, feat)
_chunk = 0


def _alloc_chunks(band_ids, per_chunk, bases, ln_of, stride=None):
    # Bands are packed so a band placed at base 32/64 always has feat >= base
    # (required for the windowed transpose: its PSUM output must start at
    # partition 0, so the input window starts `base` features early).
    # When `stride` is given, the bands sharing a chunk are spaced so that
    # their features sit exactly 32 apart — then feat_j - base_j is constant
    # across the chunk and ONE windowed transpose serves all of them.
    global _chunk
    nchunks = (len(band_ids) + per_chunk - 1) // per_chunk
    for k in range(nchunks):
        if stride is None:
            group = band_ids[k::nchunks]
        else:
            g0 = (k // stride) * per_chunk * stride + (k % stride)
            group = [band_ids[g0 + stride * j] for j in range(per_chunk)]
        for j, b in enumerate(group):
            assert OFFSETS[b] >= bases[j], (b, bases[j])
            SEGS[b].append((_chunk + k, bases[j], ln_of(b), OFFSETS[b]))
    _chunk += nchunks


_w8 = [b for b in range(NB) if BANDS[b] == 8]
_w16 = [b for b in range(NB) if BANDS[b] == 16]
_w48 = [b for b in range(NB) if BANDS[b] == 48]
_w96 = [b for b in range(NB) if BANDS[b] == 96]
_w192 = [b for b in range(NB) if BANDS[b] == 192]
_big = [b for b in range(NB) if BANDS[b] > 192]

_alloc_chunks(_w8, 3, (0, 32, 64), lambda b: 8, stride=4)   # 32/8
_alloc_chunks(_w16, 3, (0, 32, 64), lambda b: 16, stride=2)  # 32/16
_alloc_chunks(_w48, 2, (0, 64), lambda b: 48)
_alloc_chunks(_w96, 1, (0,), lambda b: 96)
# w192: one full chunk + a 64-row tail, tails packed two per chunk
for _b in _w192:
    SEGS[_b].append((_chunk, 0, 128, OFFSETS[_b]))
    _chunk += 1
for _i in range(0, len(_w192), 2):
    for _j, _b in enumerate(_w192[_i : _i + 2]):
        SEGS[_b].append((_chunk, 64 * _j, 64, OFFSETS[_b] + 128))
    _chunk += 1
# w512 / w516: full chunks (+ a 4-row tail for 516)
for _b in _big:
    _w = BANDS[_b]
    _f = OFFSETS[_b]
    while _w > 0:
        ln = min(128, _w)
        SEGS[_b].append((_chunk, 0, ln, _f))
        _chunk += 1
        _f += ln
        _w -= ln

NCHUNK = _chunk  # 49

# per chunk: transpose jobs. Slots whose window origin (feat - base) agrees
# are served by ONE windowed transpose (rows [0, base+ln) from origin).
# All jobs of a chunk share one PSUM tile: a job writes pt[0, rows), so
# issuing in descending-rows order leaves every band's rows [base, base+ln)
# intact and the whole range [0, hi) initialized -> one PSUM->SBUF copy
# per chunk. TJOBS entries are (origin_feat, rows).
CHUNK_JOBS = [[] for _ in range(NCHUNK)]
for _b in range(NB):
    for (c, base, ln, feat) in SEGS[_b]:
        CHUNK_JOBS[c].append((base, ln, feat))
CHUNK_TJOBS = []
for _jobs in CHUNK_JOBS:
    byorg = {}
    for base, ln, feat in _jobs:
        org = feat - base
        byorg[org] = max(byorg.get(org, 0), base + ln)
    CHUNK_TJOBS.append(sorted(byorg.items(), key=lambda kv: -kv[1]))
CHUNK_HI = [max(base + ln for base, ln, _ in jobs) for jobs in CHUNK_JOBS]
N_TRANSPOSES = sum(len(j) for j in CHUNK_TJOBS)  # 53 per token tile

# band segs sorted by chunk so matmuls run in chunk order
for _b in range(NB):
    SEGS[_b].sort()

# equal-width runs for grouped sum-of-squares reductions:
# (feat_start, n_bands, width, band0)
RGROUPS = [
    (0, 24, 8, 0),
    (192, 12, 16, 24),
    (384, 8, 48, 36),
    (768, 8, 96, 44),
    (1536, 8, 192, 52),
    (3072, 1, 512, 60),
    (3584, 1, 516, 61),
]

# output staging groups: bands packed per DMA (contiguous in y's band axis)
OUT_GROUP = 8
OGROUPS = [(g, min(g + OUT_GROUP, NB)) for g in range(0, NB, OUT_GROUP)]

F32 = mybir.dt.float32
# Matmul operand dtype: float32r streams at full PE rate for N>=256 (fp32
# is 4 cycles/row). Operand-producing instructions must write the dtype
# themselves (BIR verifier rule), so the wt/xc tiles are declared fp32r.
MM_DT = mybir.dt.float32r


def _build_body(ctx: ExitStack, tc: tile.TileContext, x_ap, w_ap, y_ap):
    nc = tc.nc

    const = ctx.enter_context(tc.tile_pool(name="const", bufs=1))
    wpool = ctx.enter_context(tc.tile_pool(name="wpool", bufs=1))
    xpool = ctx.enter_context(tc.tile_pool(name="xpool", bufs=2))
    x2pool = ctx.enter_context(tc.tile_pool(name="x2pool", bufs=2))
    xtpool = ctx.enter_context(tc.tile_pool(name="xtpool", bufs=6))
    stats = ctx.enter_context(tc.tile_pool(name="stats", bufs=2))
    outpool = ctx.enter_context(tc.tile_pool(name="outpool", bufs=2))
    psum_tp = ctx.enter_context(tc.tile_pool(name="psum_tp", bufs=3, space="PSUM"))
    psum_mm = ctx.enter_context(tc.tile_pool(name="psum_mm", bufs=5, space="PSUM"))

    identf = const.tile([P, P], F32, tag="identf")
    make_identity(nc, identf[:])
    ident = const.tile([P, P], MM_DT, tag="ident")
    nc.vector.tensor_copy(ident[:], identf[:])

    # weights, slot-layout in DRAM already: w_ap is (NCHUNK*128, 384);
    # SBUF wt[p, c*DIM + d] = wpad[c*128 + p, d]. Issued on the gpsimd
    # (SWDGE) queue so the bulk load streams in parallel with the
    # latency-critical first x-tile load on the sync HWDGE queue.
    wt = wpool.tile([P, NCHUNK * DIM], MM_DT, tag="wt")
    # Runs of equal-height chunks load as one DMA trimmed to rows [0, hi):
    # skips ~2.4MB of slot-padding traffic with only ~6 DMA instructions
    # (more would starve the strict-FIFO gpsimd queue and delay the
    # squares). Early chunks arrive early for the first bands' matmuls.
    _c = 0
    while _c < NCHUNK:
        hi = CHUNK_HI[_c]
        _c2 = _c
        while _c2 < NCHUNK and CHUNK_HI[_c2] == hi and _c2 - _c < 7:
            _c2 += 1
        nc.gpsimd.dma_start(
            wt[:hi, _c * DIM : _c2 * DIM].rearrange("p (c d) -> p c d", d=DIM),
            w_ap[_c * P : _c2 * P, :].rearrange("(c p) d -> p c d", p=P)[:hi],
        )
        _c = _c2

    # FS = 1536 is exactly the small-band/big-band chunk boundary. Each x
    # half loads as two parallel-queue DMAs, and the norm stats are split:
    # sca (bands 0-51) depends only on the early xta half — squared on the
    # otherwise-idle ACT — so the first evictions never wait for the late
    # half; scb rides the slower gpsimd square of xtb.
    FS = 1536
    NBA = 52
    for it in range(NTILE):
        t0 = it * P
        xta = xpool.tile([P, FS], MM_DT, tag="xta")
        nc.sync.dma_start(xta[:, : FS // 2], x_ap[t0 : t0 + P, : FS // 2])
        nc.sync.dma_start(xta[:, FS // 2 :], x_ap[t0 : t0 + P, FS // 2 : FS])
        xtb = xpool.tile([P, F - FS], MM_DT, tag="xtb")
        FM = FS + (F - FS) // 2
        nc.sync.dma_start(xtb[:, : FM - FS], x_ap[t0 : t0 + P, FS:FM])
        nc.sync.dma_start(xtb[:, FM - FS :], x_ap[t0 : t0 + P, FM:])

        x2 = x2pool.tile([P, F], F32, tag="x2")
        nc.scalar.square(x2[:, :FS], xta[:])
        qqa = stats.tile([P, NBA], F32, tag="qqa")
        for fs, nb, wd, b0 in RGROUPS:
            if b0 >= NBA:
                continue
            nc.vector.reduce_sum(
                qqa[:, b0 : b0 + nb],
                x2[:, fs : fs + nb * wd].rearrange("p (n w) -> p n w", w=wd),
                axis=mybir.AxisListType.X,
            )
        nrma = stats.tile([P, NBA], F32, tag="nrma")
        nc.scalar.sqrt(nrma[:], qqa[:])
        sca = stats.tile([P, NBA], F32, tag="sca")
        nc.vector.reciprocal(sca[:], nrma[:])

        nc.gpsimd.tensor_tensor(x2[:, FS:], xtb[:], xtb[:], op=mybir.AluOpType.mult)
        qqb = stats.tile([P, NB - NBA], F32, tag="qqb")
        for fs, nb, wd, b0 in RGROUPS:
            if b0 < NBA:
                continue
            nc.vector.reduce_sum(
                qqb[:, b0 - NBA : b0 - NBA + nb],
                x2[:, fs : fs + nb * wd].rearrange("p (n w) -> p n w", w=wd),
                axis=mybir.AxisListType.X,
            )
        nrmb = stats.tile([P, NB - NBA], F32, tag="nrmb")
        nc.scalar.sqrt(nrmb[:], qqb[:])
        scb = stats.tile([P, NB - NBA], F32, tag="scb")
        nc.vector.reciprocal(scb[:], nrmb[:])

        def xwin(org, rows):
            if org + rows <= FS:
                return xta[:, org : org + rows]
            assert org >= FS, (org, rows)
            return xtb[:, org - FS : org - FS + rows]

        # ---- transpose x into 32-aligned slot chunks: xc[base+r, t].
        # Transpose-matmul PSUM outputs must start at partition 0, so each
        # job's input window starts `base` features early; descending-base
        # issue order within the shared pt tile leaves every band's rows
        # intact and [0, hi) fully initialized -> single copy per chunk.
        # 4 chunks share one PSUM bank ([128, 512] f32) and one SBUF tile,
        # so a single wide copy evicts 4 chunks (DVE cost scales with the
        # free dim, so one [hi, 512] copy beats four [hi, 128] copies).
        xcs = [None] * NCHUNK
        xcoff = [None] * NCHUNK
        for q0 in range(0, NCHUNK, 4):
            q1 = min(q0 + 4, NCHUNK)
            xc = xtpool.tile([P, 4 * P], MM_DT, tag="xc")
            pt = psum_tp.tile([P, 4 * P], MM_DT, tag="pt")
            for c in range(q0, q1):
                col = (c - q0) * P
                for org, rows in CHUNK_TJOBS[c]:
                    nc.tensor.transpose(
                        pt[:rows, col : col + P], xwin(org, rows), ident[:]
                    )
            qhi = max(CHUNK_HI[c] for c in range(q0, q1))
            if (q0 // 4) % 2 == 0:
                nc.vector.tensor_copy(
                    xc[:qhi, : (q1 - q0) * P], pt[:qhi, : (q1 - q0) * P]
                )
            else:
                nc.scalar.copy(
                    xc[:qhi, : (q1 - q0) * P], pt[:qhi, : (q1 - q0) * P]
                )
            for c in range(q0, q1):
                xcs[c] = xc
                xcoff[c] = (c - q0) * P

        # ---- grouped GEMM + scaled eviction, bands staged in output groups
        for g0, g1 in OGROUPS:
            go = outpool.tile([P, OUT_GROUP * DIM], F32, tag="go")
            for b in range(g0, g1):
                pm = psum_mm.tile([P, DIM], F32, tag="pm")
                segs = SEGS[b]
                for si, (c, base, ln, feat) in enumerate(segs):
                    co = xcoff[c]
                    lhsT = xcs[c][base : base + ln, co : co + P]
                    rhs = wt[base : base + ln, c * DIM : (c + 1) * DIM]
                    nc.tensor.matmul(
                        pm[:],
                        lhsT,
                        rhs,
                        start=(si == 0),
                        stop=(si == len(segs) - 1),
                    )
                dst = go[:, (b - g0) * DIM : (b - g0 + 1) * DIM]
                scp = sca[:, b : b + 1] if b < NBA else scb[:, b - NBA : b - NBA + 1]
                if b % 9 < 4:  # DVE also owns copies/reduces; tilt to ACT
                    nc.vector.tensor_scalar_mul(dst, pm[:], scp)
                else:
                    nc.scalar.mul(dst, pm[:], scp)
            nc.sync.dma_start(
                y_ap[t0 : t0 + P, g0 * DIM : g1 * DIM],
                go[:, : (g1 - g0) * DIM],
            )


_cached_nc = None


def _build():
    global _cached_nc
    if _cached_nc is not None:
        return _cached_nc
    nc = bacc.Bacc(
        "TRN2",
        target_bir_lowering=False,
        debug=False,
        enable_asserts=False,
        num_devices=NCORES,
    )
    x_ap = nc.dram_tensor("x", (TOK_PER_CORE, F), MM_DT, kind="ExternalInput").ap()
    w_ap = nc.dram_tensor("w", (NCHUNK * P, DIM), MM_DT, kind="ExternalInput").ap()
    y_ap = nc.dram_tensor(
        "y", (TOK_PER_CORE, NB * DIM), F32, kind="ExternalOutput"
    ).ap()
    with tile.TileContext(nc) as tc:
        with ExitStack() as ctx:
            _build_body(ctx, tc, x_ap, w_ap, y_ap)
    nc.compile()
    _cached_nc = nc
    return nc


def _host_prep(x, gammas, weights, biases):
    x = np.ascontiguousarray(np.asarray(x, np.float32)).reshape(NTOK, F)
    wpad = np.zeros((NCHUNK * P, DIM), np.float32)
    for b in range(NB):
        g = np.asarray(gammas[b], np.float32)
        w = np.asarray(weights[b], np.float32)
        wb = w * (g * np.float32(math.sqrt(len(g))))[:, None]
        for (c, base, ln, feat) in SEGS[b]:
            lo = OFFSETS[b]
            wpad[c * P + base : c * P + base + ln] = wb[feat - lo : feat - lo + ln]
    bias = np.stack([np.asarray(bb, np.float32) for bb in biases], axis=0)
    return x, wpad, bias


def kernel(x, gammas, weights, biases, _trace=False):
    xf, wpad, bias = _host_prep(x, gammas, weights, biases)
    nc = _build()
    in_maps = [
        {"x": xf[c * TOK_PER_CORE : (c + 1) * TOK_PER_CORE], "w": wpad}
        for c in range(NCORES)
    ]
    res = run_bass_kernel_spmd(
        nc, in_maps, core_ids=list(range(NCORES)), trace=_trace
    )
    y = np.concatenate([r["y"] for r in res.results], axis=0)
    y = y.reshape(B, T, NB, DIM)
    if np.any(bias):
        y = y + bias[None, None]
    if _trace:
        return y, res
    return y
